# revision 6
# baseline (speedup 1.0000x reference)
"""MoE (DCMoe) Trainium2 kernel: expert-parallel over 8 NeuronCores.

Strategy (per spec sharding_hint): each core owns one expert's FFN weights
and processes only the tokens routed to that expert (top-2 of 8, gathered
host-side during sharding); the shared expert is sharded over its
intermediate dim M (each core computes a 256-wide slice for all tokens).
Routing (gate logits, top-k, sigmoid) runs in fp32 on host as part of the
dispatch step; the heavy FFN matmuls run on device in float32r.

Device layout: activations flow feature-on-partition / token-on-free, so
every matmul uses weights in their natural [in, out] orientation as the
stationary operand and never needs an on-device transpose.
"""
import sys
for _p in ("/opt/trn_rl_repo", "/root/.axon_site/_ro/trn_rl_repo"):
    if _p not in sys.path:
        sys.path.append(_p)

import numpy as np

E = 8
H = 1024
M = 2048
TOPK = 2
B = 2048           # tokens = 2*1024
MS = M // E        # shared-expert M slice per core = 256
NCORES = 8
HCH = H // 128     # 8 h-chunks
MB = M // 128      # 16 m-blocks
MSB = MS // 128    # 2 shared m-blocks
TS = B // 512      # 4 token tiles of 512 for the shared expert

_RUNNERS = {}      # C -> (runner-like object)


def _build(C, reps=1, act_name="Silu"):
    import concourse.bacc as bacc
    import concourse.mybir as mybir
    from concourse import tile

    DT = mybir.dt.float32r
    F32 = mybir.dt.float32
    ACT = mybir.ActivationFunctionType
    ACT_F = getattr(ACT, act_name)
    CT = C // 2  # token tile (must be >=256 for full-rate float32r)
    assert CT >= 256 and CT <= 512

    nc = bacc.Bacc("TRN2", target_bir_lowering=False, debug=False,
                   num_devices=NCORES)

    xe_d = nc.dram_tensor("xe", [HCH, 128, C], DT, kind="ExternalInput").ap()
    xt_d = nc.dram_tensor("xt", [TS, HCH, 128, 512], DT, kind="ExternalInput").ap()
    gw_d = nc.dram_tensor("gw", [MB * HCH, 128, 128], DT, kind="ExternalInput").ap()
    uw_d = nc.dram_tensor("uw", [MB * HCH, 128, 128], DT, kind="ExternalInput").ap()
    dw_d = nc.dram_tensor("dw", [HCH * MB, 128, 128], DT, kind="ExternalInput").ap()
    sgw_d = nc.dram_tensor("sgw", [MSB * HCH, 128, 128], DT, kind="ExternalInput").ap()
    suw_d = nc.dram_tensor("suw", [MSB * HCH, 128, 128], DT, kind="ExternalInput").ap()
    sdw_d = nc.dram_tensor("sdw", [HCH * MSB, 128, 128], DT, kind="ExternalInput").ap()
    ye_d = nc.dram_tensor("ye", [HCH, 128, C], F32, kind="ExternalOutput").ap()
    ys_d = nc.dram_tensor("ys", [HCH, 128, B], F32, kind="ExternalOutput").ap()

    with tile.TileContext(nc) as tc:
        def body():
            with (
                tc.tile_pool(name="xe_p", bufs=1) as xe_p,
                tc.tile_pool(name="xt_p", bufs=3) as xt_p,
                tc.tile_pool(name="w_p", bufs=2) as w_p,
                tc.tile_pool(name="dw_p", bufs=2) as dw_p,
                tc.tile_pool(name="ht_p", bufs=1) as ht_p,
                tc.tile_pool(name="sw_p", bufs=1) as sw_p,
                tc.tile_pool(name="tmp_p", bufs=3) as tmp_p,
                tc.tile_pool(name="st_p", bufs=2) as st_p,
                tc.tile_pool(name="ps_a", bufs=1, space="PSUM") as ps_a,
                tc.tile_pool(name="ps_b", bufs=2, space="PSUM") as ps_b,
            ):
                # ---- resident loads ----
                xe_sb = xe_p.tile([128, HCH * C], DT, name="xe_sb")
                nc.sync.dma_start(
                    xe_sb[:], xe_d.rearrange("a p c -> p a c"))
                sgw_sb = sw_p.tile([128, MSB * HCH * 128], DT, name="sgw_sb")
                nc.sync.dma_start(
                    sgw_sb[:], sgw_d.rearrange("g p m -> p g m"))
                suw_sb = sw_p.tile([128, MSB * HCH * 128], DT, name="suw_sb")
                nc.sync.dma_start(
                    suw_sb[:], suw_d.rearrange("g p m -> p g m"))
                sdw_sb = sw_p.tile([128, HCH * MSB * 128], DT, name="sdw_sb")
                nc.sync.dma_start(
                    sdw_sb[:], sdw_d.rearrange("g p h -> p g h"))

                ht = [ht_p.tile([128, C], DT, name=f"ht{i}", tag=f"ht{i}")
                      for i in range(MB)]

                # ---- expert phase A: g/u/h per m-block ----
                for i in range(MB):
                    gw_sb = w_p.tile([128, HCH * 128], DT, name=f"gw{i}", tag="gw")
                    nc.sync.dma_start(
                        gw_sb[:],
                        gw_d[i * HCH:(i + 1) * HCH].rearrange("a p m -> p a m"))
                    uw_sb = w_p.tile([128, HCH * 128], DT, name=f"uw{i}", tag="uw")
                    nc.sync.dma_start(
                        uw_sb[:],
                        uw_d[i * HCH:(i + 1) * HCH].rearrange("a p m -> p a m"))
                    for t in range(2):
                        psg = ps_a.tile([128, CT], F32, name=f"psg{i}_{t}", tag=f"pa{t*2}", bufs=2)
                        for a in range(HCH):
                            nc.tensor.matmul(
                                psg[:], gw_sb[:, a * 128:(a + 1) * 128],
                                xe_sb[:, a * C + t * CT: a * C + (t + 1) * CT],
                                start=(a == 0), stop=(a == HCH - 1))
                        psu = ps_a.tile([128, CT], F32, name=f"psu{i}_{t}", tag=f"pa{t*2+1}", bufs=1)
                        for a in range(HCH):
                            nc.tensor.matmul(
                                psu[:], uw_sb[:, a * 128:(a + 1) * 128],
                                xe_sb[:, a * C + t * CT: a * C + (t + 1) * CT],
                                start=(a == 0), stop=(a == HCH - 1))
                        sg = tmp_p.tile([128, CT], DT, name=f"sg{i}_{t}", tag="sg")
                        nc.scalar.activation(sg[:], psg[:], ACT_F)
                        nc.vector.tensor_mul(
                            ht[i][:, t * CT:(t + 1) * CT], sg[:], psu[:])

                # ---- expert phase B: down proj ----
                for a in range(HCH):
                    dw_sb = dw_p.tile([128, MB * 128], DT, name=f"dw{a}", tag="dw")
                    nc.sync.dma_start(
                        dw_sb[:],
                        dw_d[a * MB:(a + 1) * MB].rearrange("i p h -> p i h"))
                    ye_st = st_p.tile([128, C], F32, name=f"ye_st{a}", tag="ye_st")
                    for t in range(2):
                        psy = ps_b.tile([128, CT], F32, name=f"psy{a}_{t}", tag="pb")
                        for i in range(MB):
                            nc.tensor.matmul(
                                psy[:], dw_sb[:, i * 128:(i + 1) * 128],
                                ht[i][:, t * CT:(t + 1) * CT],
                                start=(i == 0), stop=(i == MB - 1))
                        nc.scalar.copy(ye_st[:, t * CT:(t + 1) * CT], psy[:])
                    nc.sync.dma_start(ye_d[a, :, :], ye_st[:])

                # ---- shared expert (M-slice), token tiles of 512 ----
                for ts in range(TS):
                    xt_sb = xt_p.tile([128, HCH * 512], DT, name=f"xt{ts}", tag="xt")
                    nc.sync.dma_start(
                        xt_sb[:], xt_d[ts].rearrange("a p c -> p a c"))
                    sh = []
                    for i in range(MSB):
                        psg = ps_a.tile([128, 512], F32, name=f"s_psg{ts}_{i}", tag=f"pa{i*2}", bufs=2)
                        for a in range(HCH):
                            nc.tensor.matmul(
                                psg[:],
                                sgw_sb[:, (i * HCH + a) * 128:(i * HCH + a + 1) * 128],
                                xt_sb[:, a * 512:(a + 1) * 512],
                                start=(a == 0), stop=(a == HCH - 1))
                        psu = ps_a.tile([128, 512], F32, name=f"s_psu{ts}_{i}", tag=f"pa{i*2+1}", bufs=1)
                        for a in range(HCH):
                            nc.tensor.matmul(
                                psu[:],
                                suw_sb[:, (i * HCH + a) * 128:(i * HCH + a + 1) * 128],
                                xt_sb[:, a * 512:(a + 1) * 512],
                                start=(a == 0), stop=(a == HCH - 1))
                        sgt = tmp_p.tile([128, 512], DT, name=f"s_sg{ts}_{i}", tag="s_sg")
                        nc.scalar.activation(sgt[:], psg[:], ACT_F)
                        sht = tmp_p.tile([128, 512], DT, name=f"sht{ts}_{i}", tag=f"sht{i}")
                        nc.vector.tensor_mul(sht[:], sgt[:], psu[:])
                        sh.append(sht)
                    for a in range(HCH):
                        psy = ps_b.tile([128, 512], F32, name=f"s_psy{ts}_{a}", tag="pb")
                        for i in range(MSB):
                            nc.tensor.matmul(
                                psy[:],
                                sdw_sb[:, (a * MSB + i) * 128:(a * MSB + i + 1) * 128],
                                sh[i][:],
                                start=(i == 0), stop=(i == MSB - 1))
                        ys_st = st_p.tile([128, 512], F32, name=f"ys_st{ts}_{a}", tag="ys_st")
                        if a % 2 == 0:
                            nc.scalar.copy(ys_st[:], psy[:])
                        else:
                            nc.vector.tensor_copy(ys_st[:], psy[:])
                        nc.sync.dma_start(
                            ys_d[a, :, ts * 512:(ts + 1) * 512], ys_st[:])

        if reps == 1:
            body()
        else:
            with tc.For_i(0, reps, 1):
                body()
    nc.compile()
    return nc


class _Runner:
    """Compile-once/run-many wrapper around concourse.bass2jax (axon PJRT)."""

    def __init__(self, nc):
        import jax
        import concourse.mybir as mybir
        from jax.sharding import Mesh, PartitionSpec
        from jax.experimental.shard_map import shard_map
        from concourse.bass2jax import (
            _bass_exec_p, install_neuronx_cc_hook, partition_id_tensor)

        install_neuronx_cc_hook()
        self.nc = nc
        partition_name = (nc.partition_id_tensor.name
                          if nc.partition_id_tensor else None)
        in_names, out_names, out_avals, zero_outs = [], [], [], []
        for alloc in nc.m.functions[0].allocations:
            if not isinstance(alloc, mybir.MemoryLocationSet):
                continue
            name = alloc.memorylocations[0].name
            if alloc.kind == "ExternalInput":
                if name != partition_name:
                    in_names.append(name)
            elif alloc.kind == "ExternalOutput":
                shape = tuple(alloc.tensor_shape)
                dtype = mybir.dt.np(alloc.dtype)
                out_names.append(name)
                out_avals.append(jax.core.ShapedArray(shape, dtype))
                zero_outs.append(np.zeros(shape, dtype))
        self.in_names = in_names
        self.out_names = out_names
        n_params, n_outs = len(in_names), len(out_names)
        all_in = list(in_names) + list(out_names)
        if partition_name is not None:
            all_in.append(partition_name)

        def _bdy(*args):
            operands = list(args)
            if partition_name is not None:
                operands.append(partition_id_tensor())
            return tuple(_bass_exec_p.bind(
                *operands, out_avals=tuple(out_avals), in_names=tuple(all_in),
                out_names=tuple(out_names), lowering_input_output_aliases=(),
                sim_require_finite=False, sim_require_nnan=False, nc=nc))

        devices = jax.devices()[:NCORES]
        mesh = Mesh(np.asarray(devices), ("core",))
        self._fn = jax.jit(
            shard_map(_bdy, mesh=mesh,
                      in_specs=(PartitionSpec("core"),) * (n_params + n_outs),
                      out_specs=(PartitionSpec("core"),) * n_outs,
                      check_rep=False),
            keep_unused=True)
        self._concat_zeros = [np.concatenate([z] * NCORES, axis=0)
                              for z in zero_outs]
        self._jax = jax

    def run(self, in_maps):
        jax = self._jax
        concat_in = [
            np.concatenate([np.asarray(m[name]) for m in in_maps], axis=0)
            for name in self.in_names]
        out = self._fn(*concat_in, *self._concat_zeros)
        jax.block_until_ready(out)
        results = []
        for c in range(NCORES):
            d = {}
            for i, name in enumerate(self.out_names):
                arr = np.asarray(out[i])
                per = arr.shape[0] // NCORES
                d[name] = arr[c * per:(c + 1) * per]
            results.append(d)
        return results


def _get_runner(C):
    if C not in _RUNNERS:
        _RUNNERS[C] = _Runner(_build(C))
    return _RUNNERS[C]


def _route(x2d, gate_w, beta):
    """fp32 routing exactly matching the jax reference semantics."""
    logits = x2d @ gate_w.T                              # [B, E] fp32
    sel = logits + beta[None, :]
    idx = np.argsort(-sel, axis=-1, kind="stable")[:, :TOPK]   # [B, K]
    picked = np.take_along_axis(logits, idx, axis=-1)
    probs = 1.0 / (1.0 + np.exp(-picked.astype(np.float32)))
    return idx, probs.astype(np.float32)


def _block_w(w, blocks_first=True):
    """[In, Out] -> [(out_blk, in_blk), 128_in, 128_out] contiguous blocks."""
    i_, o_ = w.shape
    r = w.reshape(i_ // 128, 128, o_ // 128, 128)
    return np.ascontiguousarray(r.transpose(2, 0, 1, 3)).reshape(-1, 128, 128)


def kernel(x, gate_w, beta, gate_proj_w, up_proj_w, down_proj_w,
           shared_gate_w, shared_up_w, shared_down_w):
    x = np.asarray(x, dtype=np.float32)
    gate_w = np.asarray(gate_w, dtype=np.float32)
    beta = np.asarray(beta, dtype=np.float32)
    gate_proj_w = np.asarray(gate_proj_w, dtype=np.float32)
    up_proj_w = np.asarray(up_proj_w, dtype=np.float32)
    down_proj_w = np.asarray(down_proj_w, dtype=np.float32)
    shared_gate_w = np.asarray(shared_gate_w, dtype=np.float32)
    shared_up_w = np.asarray(shared_up_w, dtype=np.float32)
    shared_down_w = np.asarray(shared_down_w, dtype=np.float32)

    B0, S0, H0 = x.shape
    x2d = x.reshape(-1, H0)
    nb = x2d.shape[0]
    assert nb == B and H0 == H

    idx, probs = _route(x2d, gate_w, beta)
    tok_lists = [np.where((idx == e).any(axis=1))[0] for e in range(E)]
    counts = np.array([len(t) for t in tok_lists])

    C = 576
    while counts.max() > C:
        C += 192
    runner = _get_runner(C)

    xt = np.ascontiguousarray(x2d.T)                     # [H, B]
    xt_blk = np.ascontiguousarray(
        xt.reshape(HCH, 128, TS, 512).transpose(2, 0, 1, 3))

    in_maps = []
    for c in range(E):
        toks = tok_lists[c]
        xe = np.zeros((H, C), np.float32)
        xe[:, :len(toks)] = xt[:, toks]
        in_maps.append({
            "xe": xe.reshape(HCH, 128, C),
            "xt": xt_blk,
            "gw": _block_w(gate_proj_w[c], True),
            "uw": _block_w(up_proj_w[c], True),
            "dw": _block_w(down_proj_w[c], False),
            "sgw": _block_w(shared_gate_w[0][:, c * MS:(c + 1) * MS], True),
            "suw": _block_w(shared_up_w[0][:, c * MS:(c + 1) * MS], True),
            "sdw": _block_w(shared_down_w[0][c * MS:(c + 1) * MS, :], False),
        })

    results = runner.run(in_maps)

    out = np.zeros((nb, H), np.float32)
    for c in range(E):
        toks = tok_lists[c]
        ye = results[c]["ye"].reshape(H, C)[:, :len(toks)].T   # [n_e, H]
        w = np.where(idx[toks] == c, probs[toks], 0.0).sum(axis=1)
        out[toks] += ye * w[:, None]
        out += results[c]["ys"].reshape(H, B).T
    return out.reshape(B0, S0, H0).astype(np.float32)
